# revision 6
# baseline (speedup 1.0000x reference)
"""CORLoss Trainium2 kernel (v5).

Reference (per row of N=128):
    mean1 = mean(d1) + EPS ; mean2 = mean(d2) + EPS
    std1, std2 unbiased ; cov = sum((d1-mean1)*(d2-mean2))/(n-1)
    cor  = (cov / (std1*std2 + EPS)) ** 3
    tl1  = -log((cor + 1 + EPS)/2)
    tl2  = mean(|softmax(d1) - softmax(d2)|)
    a = |cor| ; loss_row = a*tl1 + (1-a)*tl2
    out  = sum(loss_row) over all B rows, shape (1,)

Strategy: data-parallel over 8 NeuronCores, 16384 rows/core, streamed as
[128 partitions, 16 blocks, 128] supertiles (row = partition*NB + block,
8 KiB contiguous DRAM per partition per supertile).

v7: the backend prices DVE segmented reduces at 1x but tensor_tensor
at 2x (bf16) and ACT/DMA/Pool nearly free, so:
  - s1,q1,s2,q2 come from one 4-channel {d1,d2,sq1,sq2} halving tree,
    with the squares computed on ACT (bn_stats does not compile on
    this backend's neuronx-cc path);
  - se1,se2 and s12,sum_min come from bf16 halving trees folded all
    the way to width 1 (no 1x reduce tails);
  - prod = d1*d2 runs on the otherwise-idle GpSimd (Pool) engine;
  - the |g| pass is gone via  sum|e1 - c*e2| = 2*(se1 - sum(min(e1,
    c*e2)))  [c = se1/se2], so tl2 = (2/N)*(1 - sum_min/se1).

Per supertile:
  DMA (SWDGE)  fp32->bf16 cast loads of d1, d2
  Pool         prod = d1*d2 into megaT[0]
  ACT          Exp (2ch) into megaT[1:3]; Square (2ch) into DS[2:4]
  DVE          {d1,d2,sq1,sq2} tree; e-tree {e1,e2} -> se1,se2 feeding
               the c -> f -> m chain early; f = c*e2 via (c,c) bf16
               pair broadcast; m = min(e1, f); shared {prod, m} tree
  epilogue     per-row cor/tl1/tl2/loss on [128,128] stat tiles;
               one [128,1] partial per core; host adds 8*128 partials.
"""

import sys

sys.path.insert(0, "/opt/trn_rl_repo")

import numpy as np

import concourse.bass as bass
import concourse.tile as tile
from concourse import mybir

B, N = 131072, 128
EPS = 1e-3
N_CORES = 8
R = B // N_CORES          # rows per core = 16384
ST_ROWS = 2048            # rows per supertile
NB = ST_ROWS // 128       # 16 row-blocks per supertile
NST = R // ST_ROWS        # 8 supertiles per core
NCOLS = R // 128          # 128 stat columns per core
F32 = mybir.dt.float32
BF16 = mybir.dt.bfloat16
Alu = mybir.AluOpType
Act = mybir.ActivationFunctionType


def _tt(nc, out, a, b, op):
    nc.vector.tensor_tensor(out=out, in0=a, in1=b, op=op)


def split_waits(nc, cap=1):
    """This walrus build rejects instructions carrying more than ~1 inline
    semaphore wait; move excess waits onto fresh same-engine nops placed
    immediately before the instruction."""
    for fn in nc.m.functions:
        for bb in fn.blocks:
            snapshot = list(bb.instructions)
            out = []
            for inst in snapshot:
                si = inst.sync_info
                if si is not None and si.on_wait and len(si.on_wait) > cap:
                    waits = list(si.on_wait)
                    extra, keep = waits[:-cap], waits[-cap:]
                    while si.on_wait:
                        si.on_wait.pop()
                    for w in keep:
                        si.on_wait.append(w)
                    for w in extra:
                        bi = nc.engines[inst.engine].nop(nofuse=True, hint="wsplit")
                        nop_inst = bi.ins
                        for fb in nc.m.functions[0].blocks:
                            if fb.instructions and fb.instructions[-1] is nop_inst:
                                fb.instructions.pop()
                                break
                        nop_inst.sync_info = mybir.SyncInfo(on_wait=[w], on_update=[])
                        out.append(nop_inst)
                out.append(inst)
            bb.instructions[:] = out


def _build_program(loop_k=None, unroll=None):
    nc = bass.Bass()
    d1 = nc.dram_tensor("d1", [R, N], F32, kind="ExternalInput")
    d2 = nc.dram_tensor("d2", [R, N], F32, kind="ExternalInput")
    y = nc.dram_tensor("y", [128, 1], F32, kind="ExternalOutput")

    with tile.TileContext(nc) as tc:
        with (
            tc.tile_pool(name="mega", bufs=4) as mega_pool,
            tc.tile_pool(name="work", bufs=2) as work_pool,
            tc.tile_pool(name="small", bufs=2) as small_pool,
            tc.tile_pool(name="stats", bufs=1) as stats_pool,
            tc.tile_pool(name="epi", bufs=1) as epi_pool,
        ):
            # per-row statistics, one column per 128-row block
            # statsT: 0=se1 1=se2 2=s12 3=sum_min
            # statsS: 0=s1 1=s2 2=q1 3=q2
            statsT = stats_pool.tile([128, 4, NCOLS], F32, tag="statsT", name="statsT")
            statsS = stats_pool.tile([128, 4, NCOLS], F32, tag="statsS", name="statsS")

            def deep_tree(src, nch, prefix, out_col, l1_pool=False):
                """bf16 halving tree [128,nch,NB,128] -> fp32 [128,nch,NB],
                folded to width 1 (every level 2x-eligible but the last).
                l1_pool=True runs the big first fold on GpSimd."""
                t = src
                for w in (64, 32, 16, 8, 4, 2):
                    nt = work_pool.tile(
                        [128, nch, NB, w], BF16, tag=f"{prefix}{w}", name=f"{prefix}{w}"
                    )
                    if w == 64 and l1_pool:
                        nc.gpsimd.tensor_tensor(
                            out=nt, in0=t[:, :, :, 0:w], in1=t[:, :, :, w : 2 * w],
                            op=Alu.add,
                        )
                    else:
                        _tt(nc, nt, t[:, :, :, 0:w], t[:, :, :, w : 2 * w], Alu.add)
                    t = nt
                oc = out_col.rearrange("p c (b o) -> p c b o", o=1)
                _tt(nc, oc, t[:, :, :, 0:1], t[:, :, :, 1:2], Alu.add)

            def one_supertile(st):
                rows = slice(st * ST_ROWS, (st + 1) * ST_ROWS)
                cols = slice(st * NB, (st + 1) * NB)
                src1 = d1[rows, :].rearrange("(p b) n -> p b n", p=128)
                src2 = d2[rows, :].rearrange("(p b) n -> p b n", p=128)

                # DS: 0=d1 1=d2 (bf16, cast during DMA) 2=d1^2 3=d2^2 (ACT)
                DS = mega_pool.tile([128, 4, NB, N], BF16, tag="DS", name="DS")
                D = DS[:, 0:2]
                nc.gpsimd.dma_start(out=DS[:, 0], in_=src1)
                nc.gpsimd.dma_start(out=DS[:, 1], in_=src2)

                # megaT: 0=prod (Pool) 1=e1 2=e2 (ACT)
                megaT = work_pool.tile([128, 3, NB, N], BF16, tag="megaT", name="megaT")
                # exp first: it heads the long c -> f -> m -> tree chain
                nc.scalar.activation(out=megaT[:, 1:3], in_=D, func=Act.Exp)
                nc.gpsimd.tensor_tensor(out=megaT[:, 0], in0=D[:, 0], in1=D[:, 1], op=Alu.mult)

                # e-tree -> se1, se2 (releases the c chain early)
                deep_tree(megaT[:, 1:3], 2, "e", statsT[:, 0:2, cols])

                # squares on ACT (slack), then one 4ch {d1,d2,sq1,sq2} tree
                # with its big first fold on GpSimd
                nc.scalar.activation(out=DS[:, 2:4], in_=D, func=Act.Square)
                deep_tree(DS, 4, "s", statsS[:, :, cols], l1_pool=True)

                # c = se1/se2 per row of this supertile, as (c,c) bf16 pairs
                rc = small_pool.tile([128, NB], F32, tag="rc", name="rc")
                nc.vector.reciprocal(out=rc, in_=statsT[:, 1, cols])
                cst = small_pool.tile([128, NB], F32, tag="cst", name="cst")
                _tt(nc, cst, statsT[:, 0, cols], rc, Alu.mult)
                cpair = small_pool.tile([128, NB, 2], BF16, tag="cpair", name="cpair")
                nc.vector.tensor_copy(
                    out=cpair,
                    in_=cst.rearrange("p (b o) -> p b o", o=1).broadcast_to(
                        [128, NB, 2]
                    ),
                )

                # f = c*e2 in one tensor_tensor (in1 = (c,c) pairs repeated)
                f = work_pool.tile([128, NB, N], BF16, tag="f", name="f")
                cb = cpair.rearrange("p b (o two) -> p b o two", o=1).broadcast_to(
                    [128, NB, N // 2, 2]
                )
                e2v = megaT[:, 2].rearrange("p b (h two) -> p b h two", two=2)
                _tt(nc, f.rearrange("p b (h two) -> p b h two", two=2), e2v, cb, Alu.mult)
                # m = min(e1, f) ; sum|e1-f| = 2*(se1 - sum(m))
                m = work_pool.tile([128, NB, N], BF16, tag="m", name="m")
                _tt(nc, m, megaT[:, 1], f, Alu.min)

                # shared {prod, m} tree: separate L1s into one buffer,
                # then one 2-channel deep chain -> s12, sum_min
                pmL1 = work_pool.tile([128, 2, NB, 64], BF16, tag="pmL1", name="pmL1")
                nc.gpsimd.tensor_tensor(
                    out=pmL1[:, 0], in0=megaT[:, 0, :, 0:64],
                    in1=megaT[:, 0, :, 64:128], op=Alu.add,
                )
                _tt(nc, pmL1[:, 1], m[:, :, 0:64], m[:, :, 64:128], Alu.add)
                t = pmL1
                for w in (32, 16, 8, 4, 2):
                    nt = work_pool.tile([128, 2, NB, w], BF16, tag=f"pm{w}", name=f"pm{w}")
                    _tt(nc, nt, t[:, :, :, 0:w], t[:, :, :, w : 2 * w], Alu.add)
                    t = nt
                oc = statsT[:, 2:4, cols].rearrange("p c (b o) -> p c b o", o=1)
                _tt(nc, oc, t[:, :, :, 0:1], t[:, :, :, 1:2], Alu.add)

            if loop_k is not None:
                with tc.For_i(0, loop_k):
                    for st in range(NST):
                        one_supertile(st)
            elif unroll is not None:
                # python-unrolled repetitions (SWDGE DMA is not supported
                # inside For_i by this walrus build: InstIncSwdgeSem)
                for _rep in range(unroll):
                    for st in range(NST):
                        one_supertile(st)
            else:
                for st in range(NST):
                    one_supertile(st)

            # ---- per-row epilogue on [128, NCOLS] stat tiles ----
            def ep(name):
                return epi_pool.tile([128, NCOLS], F32, tag=name, name=name)

            s1a, s2a = statsS[:, 0, :], statsS[:, 1, :]
            q1a, q2a = statsS[:, 2, :], statsS[:, 3, :]
            s12a = statsT[:, 2, :]
            se1a = statsT[:, 0, :]

            # M2 = q - s^2/n ; num = s12 - s1*s2/n + n*EPS^2
            u1, m2_1 = ep("u1"), ep("m2_1")
            _tt(nc, u1, s1a, s1a, Alu.mult)
            nc.vector.scalar_tensor_tensor(
                out=m2_1, in0=u1, scalar=-1.0 / N, in1=q1a, op0=Alu.mult, op1=Alu.add
            )
            u2, m2_2 = ep("u1"), ep("m2_2")
            _tt(nc, u2, s2a, s2a, Alu.mult)
            nc.vector.scalar_tensor_tensor(
                out=m2_2, in0=u2, scalar=-1.0 / N, in1=q2a, op0=Alu.mult, op1=Alu.add
            )
            u, num, w = ep("u1"), ep("num"), ep("w")
            _tt(nc, u, s1a, s2a, Alu.mult)
            nc.vector.scalar_tensor_tensor(
                out=num, in0=u, scalar=-1.0 / N, in1=s12a, op0=Alu.mult, op1=Alu.add
            )
            _tt(nc, w, m2_1, m2_2, Alu.mult)

            # cor = (num + n*EPS^2) / (sqrt(w) + (n-1)*EPS)
            sp, den, rden, cor = ep("sp"), ep("den"), ep("rden"), ep("cor")
            nc.scalar.activation(out=sp, in_=w, func=Act.Sqrt)
            nc.vector.tensor_scalar(
                out=den,
                in0=sp,
                scalar1=(N - 1) * EPS,
                scalar2=None,
                op0=Alu.add,
            )
            nc.vector.reciprocal(out=rden, in_=den)
            nc.vector.scalar_tensor_tensor(
                out=cor,
                in0=num,
                scalar=float(N) * EPS * EPS,
                in1=rden,
                op0=Alu.add,
                op1=Alu.mult,
            )
            c2, cor3 = ep("u1"), ep("cor3")
            _tt(nc, c2, cor, cor, Alu.mult)
            _tt(nc, cor3, c2, cor, Alu.mult)

            aa, lg, tl1 = ep("aa"), ep("lg"), ep("tl1")
            ln_bias = epi_pool.tile([128, 1], F32, tag="ln_bias", name="ln_bias")
            nc.vector.memset(ln_bias, 1.0 + EPS)
            nc.vector.scalar_tensor_tensor(
                out=aa, in0=cor3, scalar=-1.0, in1=cor3, op0=Alu.mult, op1=Alu.max
            )
            nc.scalar.activation(out=lg, in_=cor3, func=Act.Ln, bias=ln_bias)
            nc.vector.tensor_scalar(
                out=tl1,
                in0=lg,
                scalar1=-1.0,
                scalar2=float(np.log(2.0)),
                op0=Alu.mult,
                op1=Alu.add,
            )
            # tl2 = (2/N)*(1 - sum_min/se1)
            r1, um, tl2 = ep("r1"), ep("um"), ep("tl2")
            nc.vector.reciprocal(out=r1, in_=se1a)
            _tt(nc, um, statsT[:, 3, :], r1, Alu.mult)
            nc.vector.tensor_scalar(
                out=tl2,
                in0=um,
                scalar1=-2.0 / N,
                scalar2=2.0 / N,
                op0=Alu.mult,
                op1=Alu.add,
            )
            dd, pp, loss = ep("u1"), ep("pp"), ep("loss")
            _tt(nc, dd, tl1, tl2, Alu.subtract)
            _tt(nc, pp, aa, dd, Alu.mult)
            _tt(nc, loss, tl2, pp, Alu.add)

            part = epi_pool.tile([128, 1], F32, tag="part", name="part")
            nc.vector.reduce_sum(out=part, in_=loss, axis=mybir.AxisListType.X)
            nc.sync.dma_start(out=y[:, :], in_=part)

    split_waits(nc)
    return nc


_NC = None
_RUNNER = None


def _get_nc():
    global _NC
    if _NC is None:
        _NC = _build_program()
    return _NC


def _get_runner():
    """Compile the 8-core pjrt executable once and reuse across calls."""
    global _RUNNER
    if _RUNNER is not None:
        return _RUNNER
    import jax
    from jax.sharding import Mesh, PartitionSpec
    from jax.experimental.shard_map import shard_map
    from concourse.bass2jax import (
        _bass_exec_p,
        install_neuronx_cc_hook,
        partition_id_tensor,
    )

    install_neuronx_cc_hook()
    nc = _get_nc()
    partition_name = nc.partition_id_tensor.name if nc.partition_id_tensor else None
    in_names, out_names, out_avals, zero_outs = [], [], [], []
    for alloc in nc.m.functions[0].allocations:
        if not isinstance(alloc, mybir.MemoryLocationSet):
            continue
        name = alloc.memorylocations[0].name
        if alloc.kind == "ExternalInput":
            if name != partition_name:
                in_names.append(name)
        elif alloc.kind == "ExternalOutput":
            out_names.append(name)
            shape = tuple(alloc.tensor_shape)
            dtype = mybir.dt.np(alloc.dtype)
            out_avals.append(jax.core.ShapedArray(shape, dtype))
            zero_outs.append(np.zeros(shape, dtype))
    n_params = len(in_names)
    all_in_names = list(in_names) + out_names
    if partition_name is not None:
        all_in_names.append(partition_name)

    def _body(*args):
        operands = list(args)
        if partition_name is not None:
            operands.append(partition_id_tensor())
        outs = _bass_exec_p.bind(
            *operands,
            out_avals=tuple(out_avals),
            in_names=tuple(all_in_names),
            out_names=tuple(out_names),
            lowering_input_output_aliases=(),
            sim_require_finite=True,
            sim_require_nnan=True,
            nc=nc,
        )
        return tuple(outs)

    devices = jax.devices()[:N_CORES]
    mesh = Mesh(np.asarray(devices), ("core",))
    n_outs = len(out_names)
    in_specs = (PartitionSpec("core"),) * (n_params + n_outs)
    out_specs = (PartitionSpec("core"),) * n_outs
    sharded = jax.jit(
        shard_map(
            _body, mesh=mesh, in_specs=in_specs, out_specs=out_specs,
            check_rep=False,
        ),
        keep_unused=True,
    )
    zero_cat = [
        np.zeros((N_CORES * z.shape[0], *z.shape[1:]), z.dtype) for z in zero_outs
    ]

    def run(d1, d2):
        ins = {"d1": d1, "d2": d2}
        out = sharded(*(ins[nm] for nm in in_names), *zero_cat)
        y = np.asarray(out[out_names.index("y")])
        return y

    _RUNNER = run
    return _RUNNER


def kernel(distribution1, distribution2):
    d1 = np.ascontiguousarray(np.asarray(distribution1, dtype=np.float32))
    d2 = np.ascontiguousarray(np.asarray(distribution2, dtype=np.float32))
    assert d1.shape == (B, N) and d2.shape == (B, N)
    y = _get_runner()(d1, d2)  # [N_CORES*128, 1] partial sums
    return np.asarray([np.sum(y.astype(np.float64))], dtype=np.float32)


# revision 7
# speedup vs baseline: 1.0844x; 1.0844x over previous
"""CORLoss Trainium2 kernel (v5).

Reference (per row of N=128):
    mean1 = mean(d1) + EPS ; mean2 = mean(d2) + EPS
    std1, std2 unbiased ; cov = sum((d1-mean1)*(d2-mean2))/(n-1)
    cor  = (cov / (std1*std2 + EPS)) ** 3
    tl1  = -log((cor + 1 + EPS)/2)
    tl2  = mean(|softmax(d1) - softmax(d2)|)
    a = |cor| ; loss_row = a*tl1 + (1-a)*tl2
    out  = sum(loss_row) over all B rows, shape (1,)

Strategy: data-parallel over 8 NeuronCores, 16384 rows/core, streamed as
[128 partitions, 16 blocks, 128] supertiles (row = partition*NB + block,
8 KiB contiguous DRAM per partition per supertile).

v7: the backend prices DVE segmented reduces at 1x but tensor_tensor
at 2x (bf16) and ACT/DMA/Pool nearly free, so:
  - s1,q1,s2,q2 come from one 4-channel {d1,d2,sq1,sq2} halving tree,
    with the squares computed on ACT (bn_stats does not compile on
    this backend's neuronx-cc path);
  - se1,se2 and s12,sum_min come from bf16 halving trees folded all
    the way to width 1 (no 1x reduce tails);
  - prod = d1*d2 runs on the otherwise-idle GpSimd (Pool) engine;
  - the |g| pass is gone via  sum|e1 - c*e2| = 2*(se1 - sum(min(e1,
    c*e2)))  [c = se1/se2], so tl2 = (2/N)*(1 - sum_min/se1).

Per supertile:
  DMA (SWDGE)  fp32->bf16 cast loads of d1, d2
  Pool         prod = d1*d2 into megaT[0]
  ACT          Exp (2ch) into megaT[1:3]; Square (2ch) into DS[2:4]
  DVE          {d1,d2,sq1,sq2} tree; e-tree {e1,e2} -> se1,se2 feeding
               the c -> f -> m chain early; f = c*e2 via (c,c) bf16
               pair broadcast; m = min(e1, f); shared {prod, m} tree
  epilogue     per-row cor/tl1/tl2/loss on [128,128] stat tiles;
               one [128,1] partial per core; host adds 8*128 partials.
"""

import sys

sys.path.insert(0, "/opt/trn_rl_repo")

import numpy as np

import concourse.bass as bass
import concourse.tile as tile
from concourse import mybir

B, N = 131072, 128
EPS = 1e-3
N_CORES = 8
R = B // N_CORES          # rows per core = 16384
ST_ROWS = 2048            # rows per supertile
NB = ST_ROWS // 128       # 16 row-blocks per supertile
NST = R // ST_ROWS        # 8 supertiles per core
NCOLS = R // 128          # 128 stat columns per core
F32 = mybir.dt.float32
BF16 = mybir.dt.bfloat16
Alu = mybir.AluOpType
Act = mybir.ActivationFunctionType


def _tt(nc, out, a, b, op):
    nc.vector.tensor_tensor(out=out, in0=a, in1=b, op=op)


def split_waits(nc, cap=1):
    """This walrus build rejects instructions carrying more than ~1 inline
    semaphore wait; move excess waits onto fresh same-engine nops placed
    immediately before the instruction."""
    for fn in nc.m.functions:
        for bb in fn.blocks:
            snapshot = list(bb.instructions)
            out = []
            for inst in snapshot:
                si = inst.sync_info
                if si is not None and si.on_wait and len(si.on_wait) > cap:
                    waits = list(si.on_wait)
                    extra, keep = waits[:-cap], waits[-cap:]
                    while si.on_wait:
                        si.on_wait.pop()
                    for w in keep:
                        si.on_wait.append(w)
                    for w in extra:
                        bi = nc.engines[inst.engine].nop(nofuse=True, hint="wsplit")
                        nop_inst = bi.ins
                        for fb in nc.m.functions[0].blocks:
                            if fb.instructions and fb.instructions[-1] is nop_inst:
                                fb.instructions.pop()
                                break
                        nop_inst.sync_info = mybir.SyncInfo(on_wait=[w], on_update=[])
                        out.append(nop_inst)
                out.append(inst)
            bb.instructions[:] = out


def _build_program(loop_k=None, unroll=None):
    nc = bass.Bass()
    d1 = nc.dram_tensor("d1", [R, N], F32, kind="ExternalInput")
    d2 = nc.dram_tensor("d2", [R, N], F32, kind="ExternalInput")
    y = nc.dram_tensor("y", [128, 1], F32, kind="ExternalOutput")

    with tile.TileContext(nc) as tc:
        with (
            tc.tile_pool(name="mega", bufs=4) as mega_pool,
            tc.tile_pool(name="work", bufs=2) as work_pool,
            tc.tile_pool(name="small", bufs=2) as small_pool,
            tc.tile_pool(name="stats", bufs=1) as stats_pool,
            tc.tile_pool(name="epi", bufs=1) as epi_pool,
        ):
            # per-row statistics, one column per 128-row block
            # statsT: 0=se1 1=se2 2=s12 3=sum_min
            # statsS: 0=s1 1=s2 2=q1 3=q2
            statsT = stats_pool.tile([128, 4, NCOLS], F32, tag="statsT", name="statsT")
            statsS = stats_pool.tile([128, 4, NCOLS], F32, tag="statsS", name="statsS")

            def deep_tree(src, nch, prefix, out_col, l1_pool=False):
                """bf16 halving tree [128,nch,NB,128] -> fp32 [128,nch,NB],
                folded to width 1 (every level 2x-eligible but the last).
                l1_pool=True runs the big first fold on GpSimd."""
                t = src
                for w in (64, 32, 16, 8, 4, 2):
                    nt = work_pool.tile(
                        [128, nch, NB, w], BF16, tag=f"{prefix}{w}", name=f"{prefix}{w}"
                    )
                    if w == 64 and l1_pool:
                        nc.gpsimd.tensor_tensor(
                            out=nt, in0=t[:, :, :, 0:w], in1=t[:, :, :, w : 2 * w],
                            op=Alu.add,
                        )
                    else:
                        _tt(nc, nt, t[:, :, :, 0:w], t[:, :, :, w : 2 * w], Alu.add)
                    t = nt
                oc = out_col.rearrange("p c (b o) -> p c b o", o=1)
                _tt(nc, oc, t[:, :, :, 0:1], t[:, :, :, 1:2], Alu.add)

            def one_supertile(st):
                rows = slice(st * ST_ROWS, (st + 1) * ST_ROWS)
                cols = slice(st * NB, (st + 1) * NB)
                src1 = d1[rows, :].rearrange("(p b) n -> p b n", p=128)
                src2 = d2[rows, :].rearrange("(p b) n -> p b n", p=128)

                # DS: 0=d1 1=d2 (bf16, cast during DMA) 2=d1^2 3=d2^2 (ACT)
                DS = mega_pool.tile([128, 4, NB, N], BF16, tag="DS", name="DS")
                D = DS[:, 0:2]
                nc.gpsimd.dma_start(out=DS[:, 0], in_=src1)
                nc.gpsimd.dma_start(out=DS[:, 1], in_=src2)

                # megaT: 0=prod (Pool) 1=e1 2=e2 (ACT)
                megaT = work_pool.tile([128, 3, NB, N], BF16, tag="megaT", name="megaT")
                # exp first: it heads the long c -> f -> m -> tree chain
                nc.scalar.activation(out=megaT[:, 1:3], in_=D, func=Act.Exp)
                nc.gpsimd.tensor_tensor(out=megaT[:, 0], in0=D[:, 0], in1=D[:, 1], op=Alu.mult)

                # e-tree -> se1, se2 (releases the c chain early)
                deep_tree(megaT[:, 1:3], 2, "e", statsT[:, 0:2, cols])

                # squares on ACT (slack), then one 4ch {d1,d2,sq1,sq2} tree
                # with its big first fold on GpSimd
                nc.scalar.activation(out=DS[:, 2:4], in_=D, func=Act.Square)
                deep_tree(DS, 4, "s", statsS[:, :, cols])

                # c = se1/se2 per row of this supertile, as (c,c) bf16 pairs
                rc = small_pool.tile([128, NB], F32, tag="rc", name="rc")
                nc.vector.reciprocal(out=rc, in_=statsT[:, 1, cols])
                cst = small_pool.tile([128, NB], F32, tag="cst", name="cst")
                _tt(nc, cst, statsT[:, 0, cols], rc, Alu.mult)
                cpair = small_pool.tile([128, NB, 2], BF16, tag="cpair", name="cpair")
                nc.vector.tensor_copy(
                    out=cpair,
                    in_=cst.rearrange("p (b o) -> p b o", o=1).broadcast_to(
                        [128, NB, 2]
                    ),
                )

                # f = c*e2 in one tensor_tensor (in1 = (c,c) pairs repeated)
                f = work_pool.tile([128, NB, N], BF16, tag="f", name="f")
                cb = cpair.rearrange("p b (o two) -> p b o two", o=1).broadcast_to(
                    [128, NB, N // 2, 2]
                )
                e2v = megaT[:, 2].rearrange("p b (h two) -> p b h two", two=2)
                _tt(nc, f.rearrange("p b (h two) -> p b h two", two=2), e2v, cb, Alu.mult)
                # m = min(e1, f) ; sum|e1-f| = 2*(se1 - sum(m))
                m = work_pool.tile([128, NB, N], BF16, tag="m", name="m")
                _tt(nc, m, megaT[:, 1], f, Alu.min)

                # shared {prod, m} tree: separate L1s into one buffer,
                # then one 2-channel deep chain -> s12, sum_min
                pmL1 = work_pool.tile([128, 2, NB, 64], BF16, tag="pmL1", name="pmL1")
                _tt(nc, pmL1[:, 0], megaT[:, 0, :, 0:64], megaT[:, 0, :, 64:128], Alu.add)
                _tt(nc, pmL1[:, 1], m[:, :, 0:64], m[:, :, 64:128], Alu.add)
                t = pmL1
                for w in (32, 16, 8, 4, 2):
                    nt = work_pool.tile([128, 2, NB, w], BF16, tag=f"pm{w}", name=f"pm{w}")
                    _tt(nc, nt, t[:, :, :, 0:w], t[:, :, :, w : 2 * w], Alu.add)
                    t = nt
                oc = statsT[:, 2:4, cols].rearrange("p c (b o) -> p c b o", o=1)
                _tt(nc, oc, t[:, :, :, 0:1], t[:, :, :, 1:2], Alu.add)

            if loop_k is not None:
                with tc.For_i(0, loop_k):
                    for st in range(NST):
                        one_supertile(st)
            elif unroll is not None:
                # python-unrolled repetitions (SWDGE DMA is not supported
                # inside For_i by this walrus build: InstIncSwdgeSem)
                for _rep in range(unroll):
                    for st in range(NST):
                        one_supertile(st)
            else:
                for st in range(NST):
                    one_supertile(st)

            # ---- per-row epilogue on [128, NCOLS] stat tiles ----
            def ep(name):
                return epi_pool.tile([128, NCOLS], F32, tag=name, name=name)

            s1a, s2a = statsS[:, 0, :], statsS[:, 1, :]
            q1a, q2a = statsS[:, 2, :], statsS[:, 3, :]
            s12a = statsT[:, 2, :]
            se1a = statsT[:, 0, :]

            # M2 = q - s^2/n ; num = s12 - s1*s2/n + n*EPS^2
            u1, m2_1 = ep("u1"), ep("m2_1")
            _tt(nc, u1, s1a, s1a, Alu.mult)
            nc.vector.scalar_tensor_tensor(
                out=m2_1, in0=u1, scalar=-1.0 / N, in1=q1a, op0=Alu.mult, op1=Alu.add
            )
            u2, m2_2 = ep("u1"), ep("m2_2")
            _tt(nc, u2, s2a, s2a, Alu.mult)
            nc.vector.scalar_tensor_tensor(
                out=m2_2, in0=u2, scalar=-1.0 / N, in1=q2a, op0=Alu.mult, op1=Alu.add
            )
            u, num, w = ep("u1"), ep("num"), ep("w")
            _tt(nc, u, s1a, s2a, Alu.mult)
            nc.vector.scalar_tensor_tensor(
                out=num, in0=u, scalar=-1.0 / N, in1=s12a, op0=Alu.mult, op1=Alu.add
            )
            _tt(nc, w, m2_1, m2_2, Alu.mult)

            # cor = (num + n*EPS^2) / (sqrt(w) + (n-1)*EPS)
            sp, den, rden, cor = ep("sp"), ep("den"), ep("rden"), ep("cor")
            nc.scalar.activation(out=sp, in_=w, func=Act.Sqrt)
            nc.vector.tensor_scalar(
                out=den,
                in0=sp,
                scalar1=(N - 1) * EPS,
                scalar2=None,
                op0=Alu.add,
            )
            nc.vector.reciprocal(out=rden, in_=den)
            nc.vector.scalar_tensor_tensor(
                out=cor,
                in0=num,
                scalar=float(N) * EPS * EPS,
                in1=rden,
                op0=Alu.add,
                op1=Alu.mult,
            )
            c2, cor3 = ep("u1"), ep("cor3")
            _tt(nc, c2, cor, cor, Alu.mult)
            _tt(nc, cor3, c2, cor, Alu.mult)

            aa, lg, tl1 = ep("aa"), ep("lg"), ep("tl1")
            ln_bias = epi_pool.tile([128, 1], F32, tag="ln_bias", name="ln_bias")
            nc.vector.memset(ln_bias, 1.0 + EPS)
            nc.vector.scalar_tensor_tensor(
                out=aa, in0=cor3, scalar=-1.0, in1=cor3, op0=Alu.mult, op1=Alu.max
            )
            nc.scalar.activation(out=lg, in_=cor3, func=Act.Ln, bias=ln_bias)
            nc.vector.tensor_scalar(
                out=tl1,
                in0=lg,
                scalar1=-1.0,
                scalar2=float(np.log(2.0)),
                op0=Alu.mult,
                op1=Alu.add,
            )
            # tl2 = (2/N)*(1 - sum_min/se1)
            r1, um, tl2 = ep("r1"), ep("um"), ep("tl2")
            nc.vector.reciprocal(out=r1, in_=se1a)
            _tt(nc, um, statsT[:, 3, :], r1, Alu.mult)
            nc.vector.tensor_scalar(
                out=tl2,
                in0=um,
                scalar1=-2.0 / N,
                scalar2=2.0 / N,
                op0=Alu.mult,
                op1=Alu.add,
            )
            dd, pp, loss = ep("u1"), ep("pp"), ep("loss")
            _tt(nc, dd, tl1, tl2, Alu.subtract)
            _tt(nc, pp, aa, dd, Alu.mult)
            _tt(nc, loss, tl2, pp, Alu.add)

            part = epi_pool.tile([128, 1], F32, tag="part", name="part")
            nc.vector.reduce_sum(out=part, in_=loss, axis=mybir.AxisListType.X)
            nc.sync.dma_start(out=y[:, :], in_=part)

    split_waits(nc)
    return nc


_NC = None
_RUNNER = None


def _get_nc():
    global _NC
    if _NC is None:
        _NC = _build_program()
    return _NC


def _get_runner():
    """Compile the 8-core pjrt executable once and reuse across calls."""
    global _RUNNER
    if _RUNNER is not None:
        return _RUNNER
    import jax
    from jax.sharding import Mesh, PartitionSpec
    from jax.experimental.shard_map import shard_map
    from concourse.bass2jax import (
        _bass_exec_p,
        install_neuronx_cc_hook,
        partition_id_tensor,
    )

    install_neuronx_cc_hook()
    nc = _get_nc()
    partition_name = nc.partition_id_tensor.name if nc.partition_id_tensor else None
    in_names, out_names, out_avals, zero_outs = [], [], [], []
    for alloc in nc.m.functions[0].allocations:
        if not isinstance(alloc, mybir.MemoryLocationSet):
            continue
        name = alloc.memorylocations[0].name
        if alloc.kind == "ExternalInput":
            if name != partition_name:
                in_names.append(name)
        elif alloc.kind == "ExternalOutput":
            out_names.append(name)
            shape = tuple(alloc.tensor_shape)
            dtype = mybir.dt.np(alloc.dtype)
            out_avals.append(jax.core.ShapedArray(shape, dtype))
            zero_outs.append(np.zeros(shape, dtype))
    n_params = len(in_names)
    all_in_names = list(in_names) + out_names
    if partition_name is not None:
        all_in_names.append(partition_name)

    def _body(*args):
        operands = list(args)
        if partition_name is not None:
            operands.append(partition_id_tensor())
        outs = _bass_exec_p.bind(
            *operands,
            out_avals=tuple(out_avals),
            in_names=tuple(all_in_names),
            out_names=tuple(out_names),
            lowering_input_output_aliases=(),
            sim_require_finite=True,
            sim_require_nnan=True,
            nc=nc,
        )
        return tuple(outs)

    devices = jax.devices()[:N_CORES]
    mesh = Mesh(np.asarray(devices), ("core",))
    n_outs = len(out_names)
    in_specs = (PartitionSpec("core"),) * (n_params + n_outs)
    out_specs = (PartitionSpec("core"),) * n_outs
    sharded = jax.jit(
        shard_map(
            _body, mesh=mesh, in_specs=in_specs, out_specs=out_specs,
            check_rep=False,
        ),
        keep_unused=True,
    )
    zero_cat = [
        np.zeros((N_CORES * z.shape[0], *z.shape[1:]), z.dtype) for z in zero_outs
    ]

    def run(d1, d2):
        ins = {"d1": d1, "d2": d2}
        out = sharded(*(ins[nm] for nm in in_names), *zero_cat)
        y = np.asarray(out[out_names.index("y")])
        return y

    _RUNNER = run
    return _RUNNER


def kernel(distribution1, distribution2):
    d1 = np.ascontiguousarray(np.asarray(distribution1, dtype=np.float32))
    d2 = np.ascontiguousarray(np.asarray(distribution2, dtype=np.float32))
    assert d1.shape == (B, N) and d2.shape == (B, N)
    y = _get_runner()(d1, d2)  # [N_CORES*128, 1] partial sums
    return np.asarray([np.sum(y.astype(np.float64))], dtype=np.float32)


# revision 8
# speedup vs baseline: 1.6021x; 1.4774x over previous
"""CORLoss Trainium2 kernel (v5).

Reference (per row of N=128):
    mean1 = mean(d1) + EPS ; mean2 = mean(d2) + EPS
    std1, std2 unbiased ; cov = sum((d1-mean1)*(d2-mean2))/(n-1)
    cor  = (cov / (std1*std2 + EPS)) ** 3
    tl1  = -log((cor + 1 + EPS)/2)
    tl2  = mean(|softmax(d1) - softmax(d2)|)
    a = |cor| ; loss_row = a*tl1 + (1-a)*tl2
    out  = sum(loss_row) over all B rows, shape (1,)

Strategy: data-parallel over 8 NeuronCores, 16384 rows/core, streamed as
[128 partitions, 16 blocks, 128] supertiles (row = partition*NB + block,
8 KiB contiguous DRAM per partition per supertile).

v7: the backend prices DVE segmented reduces at 1x but tensor_tensor
at 2x (bf16) and ACT/DMA/Pool nearly free, so:
  - s1,q1,s2,q2 come from one 4-channel {d1,d2,sq1,sq2} halving tree,
    with the squares computed on ACT (bn_stats does not compile on
    this backend's neuronx-cc path);
  - se1,se2 and s12,sum_min come from bf16 halving trees folded all
    the way to width 1 (no 1x reduce tails);
  - prod = d1*d2 runs on the otherwise-idle GpSimd (Pool) engine;
  - the |g| pass is gone via  sum|e1 - c*e2| = 2*(se1 - sum(min(e1,
    c*e2)))  [c = se1/se2], so tl2 = (2/N)*(1 - sum_min/se1).

Per supertile:
  DMA (SWDGE)  fp32->bf16 cast loads of d1, d2
  Pool         prod = d1*d2 into megaT[0]
  ACT          Exp (2ch) into megaT[1:3]; Square (2ch) into DS[2:4]
  DVE          {d1,d2,sq1,sq2} tree; e-tree {e1,e2} -> se1,se2 feeding
               the c -> f -> m chain early; f = c*e2 via (c,c) bf16
               pair broadcast; m = min(e1, f); shared {prod, m} tree
  epilogue     per-row cor/tl1/tl2/loss on [128,128] stat tiles;
               one [128,1] partial per core; host adds 8*128 partials.
"""

import sys

sys.path.insert(0, "/opt/trn_rl_repo")

import numpy as np

import concourse.bass as bass
import concourse.tile as tile
from concourse import mybir

B, N = 131072, 128
EPS = 1e-3
N_CORES = 8
R = B // N_CORES          # rows per core = 16384
ST_ROWS = 2048            # rows per supertile
NB = ST_ROWS // 128       # 16 row-blocks per supertile
NST = R // ST_ROWS        # 8 supertiles per core
NCOLS = R // 128          # 128 stat columns per core
F32 = mybir.dt.float32
BF16 = mybir.dt.bfloat16
Alu = mybir.AluOpType
Act = mybir.ActivationFunctionType


def _tt(nc, out, a, b, op):
    nc.vector.tensor_tensor(out=out, in0=a, in1=b, op=op)


def split_waits(nc, cap=1):
    """This walrus build rejects instructions carrying more than ~1 inline
    semaphore wait; move excess waits onto fresh same-engine nops placed
    immediately before the instruction."""
    for fn in nc.m.functions:
        for bb in fn.blocks:
            snapshot = list(bb.instructions)
            out = []
            for inst in snapshot:
                si = inst.sync_info
                if si is not None and si.on_wait and len(si.on_wait) > cap:
                    waits = list(si.on_wait)
                    extra, keep = waits[:-cap], waits[-cap:]
                    while si.on_wait:
                        si.on_wait.pop()
                    for w in keep:
                        si.on_wait.append(w)
                    for w in extra:
                        bi = nc.engines[inst.engine].nop(nofuse=True, hint="wsplit")
                        nop_inst = bi.ins
                        for fb in nc.m.functions[0].blocks:
                            if fb.instructions and fb.instructions[-1] is nop_inst:
                                fb.instructions.pop()
                                break
                        nop_inst.sync_info = mybir.SyncInfo(on_wait=[w], on_update=[])
                        out.append(nop_inst)
                out.append(inst)
            bb.instructions[:] = out


def _build_program(loop_k=None, unroll=None):
    nc = bass.Bass()
    d1 = nc.dram_tensor("d1", [R, N], F32, kind="ExternalInput")
    d2 = nc.dram_tensor("d2", [R, N], F32, kind="ExternalInput")
    y = nc.dram_tensor("y", [128, 1], F32, kind="ExternalOutput")

    with tile.TileContext(nc) as tc:
        with (
            tc.tile_pool(name="mega", bufs=4) as mega_pool,
            tc.tile_pool(name="work", bufs=2) as work_pool,
            tc.tile_pool(name="small", bufs=2) as small_pool,
            tc.tile_pool(name="stats", bufs=1) as stats_pool,
            tc.tile_pool(name="epi", bufs=1) as epi_pool,
        ):
            # per-row statistics, one column per 128-row block
            # statsT: 0=se1 1=se2 2=s12 3=sum_min
            # statsS: 0=s1 1=s2 2=q1 3=q2
            statsT = stats_pool.tile([128, 4, NCOLS], F32, tag="statsT", name="statsT")
            statsS = stats_pool.tile([128, 4, NCOLS], F32, tag="statsS", name="statsS")

            def deep_tree(src, nch, prefix, out_col):
                """bf16 halving tree [128,nch,NB,128] -> 16 wide (2x mode),
                then one fp32 segmented reduce -> [128,nch,NB]."""
                t = src
                for w in (64, 32, 16):
                    nt = work_pool.tile(
                        [128, nch, NB, w], BF16, tag=f"{prefix}{w}", name=f"{prefix}{w}"
                    )
                    _tt(nc, nt, t[:, :, :, 0:w], t[:, :, :, w : 2 * w], Alu.add)
                    t = nt
                nc.vector.reduce_sum(out=out_col, in_=t, axis=mybir.AxisListType.X)

            def one_supertile(st):
                rows = slice(st * ST_ROWS, (st + 1) * ST_ROWS)
                cols = slice(st * NB, (st + 1) * NB)
                src1 = d1[rows, :].rearrange("(p b) n -> p b n", p=128)
                src2 = d2[rows, :].rearrange("(p b) n -> p b n", p=128)

                # DS: 0=d1 1=d2 (bf16, cast during DMA) 2=d1^2 3=d2^2 (ACT)
                DS = mega_pool.tile([128, 4, NB, N], BF16, tag="DS", name="DS")
                D = DS[:, 0:2]
                nc.gpsimd.dma_start(out=DS[:, 0], in_=src1)
                nc.gpsimd.dma_start(out=DS[:, 1], in_=src2)

                # megaT: 0=prod (Pool) 1=e1 2=e2 (ACT)
                megaT = work_pool.tile([128, 3, NB, N], BF16, tag="megaT", name="megaT")
                # exp first: it heads the long c -> f -> m -> tree chain
                nc.scalar.activation(out=megaT[:, 1:3], in_=D, func=Act.Exp)
                nc.gpsimd.tensor_tensor(out=megaT[:, 0], in0=D[:, 0], in1=D[:, 1], op=Alu.mult)

                # e-tree -> se1, se2 (releases the c chain early)
                deep_tree(megaT[:, 1:3], 2, "e", statsT[:, 0:2, cols])

                # squares on ACT (slack), then one 4ch {d1,d2,sq1,sq2} tree
                # with its big first fold on GpSimd
                nc.scalar.activation(out=DS[:, 2:4], in_=D, func=Act.Square)
                deep_tree(DS, 4, "s", statsS[:, :, cols])

                # c = se1/se2 per row of this supertile, as (c,c) bf16 pairs
                rc = small_pool.tile([128, NB], F32, tag="rc", name="rc")
                nc.vector.reciprocal(out=rc, in_=statsT[:, 1, cols])
                cst = small_pool.tile([128, NB], F32, tag="cst", name="cst")
                _tt(nc, cst, statsT[:, 0, cols], rc, Alu.mult)
                cpair = small_pool.tile([128, NB, 2], BF16, tag="cpair", name="cpair")
                nc.vector.tensor_copy(
                    out=cpair,
                    in_=cst.rearrange("p (b o) -> p b o", o=1).broadcast_to(
                        [128, NB, 2]
                    ),
                )

                # f = c*e2 in one tensor_tensor (in1 = (c,c) pairs repeated)
                f = work_pool.tile([128, NB, N], BF16, tag="f", name="f")
                cb = cpair.rearrange("p b (o two) -> p b o two", o=1).broadcast_to(
                    [128, NB, N // 2, 2]
                )
                e2v = megaT[:, 2].rearrange("p b (h two) -> p b h two", two=2)
                _tt(nc, f.rearrange("p b (h two) -> p b h two", two=2), e2v, cb, Alu.mult)
                # m = min(e1, f) ; sum|e1-f| = 2*(se1 - sum(m))
                m = work_pool.tile([128, NB, N], BF16, tag="m", name="m")
                _tt(nc, m, megaT[:, 1], f, Alu.min)

                # shared {prod, m} tree: separate L1s into one buffer,
                # then one 2-channel deep chain -> s12, sum_min
                pmL1 = work_pool.tile([128, 2, NB, 64], BF16, tag="pmL1", name="pmL1")
                _tt(nc, pmL1[:, 0], megaT[:, 0, :, 0:64], megaT[:, 0, :, 64:128], Alu.add)
                _tt(nc, pmL1[:, 1], m[:, :, 0:64], m[:, :, 64:128], Alu.add)
                t = pmL1
                for w in (32, 16):
                    nt = work_pool.tile([128, 2, NB, w], BF16, tag=f"pm{w}", name=f"pm{w}")
                    _tt(nc, nt, t[:, :, :, 0:w], t[:, :, :, w : 2 * w], Alu.add)
                    t = nt
                nc.vector.reduce_sum(
                    out=statsT[:, 2:4, cols], in_=t, axis=mybir.AxisListType.X
                )

            if loop_k is not None:
                with tc.For_i(0, loop_k):
                    for st in range(NST):
                        one_supertile(st)
            elif unroll is not None:
                # python-unrolled repetitions (SWDGE DMA is not supported
                # inside For_i by this walrus build: InstIncSwdgeSem)
                for _rep in range(unroll):
                    for st in range(NST):
                        one_supertile(st)
            else:
                for st in range(NST):
                    one_supertile(st)

            # ---- per-row epilogue on [128, NCOLS] stat tiles ----
            def ep(name):
                return epi_pool.tile([128, NCOLS], F32, tag=name, name=name)

            s1a, s2a = statsS[:, 0, :], statsS[:, 1, :]
            q1a, q2a = statsS[:, 2, :], statsS[:, 3, :]
            s12a = statsT[:, 2, :]
            se1a = statsT[:, 0, :]

            # M2 = q - s^2/n ; num = s12 - s1*s2/n + n*EPS^2
            u1, m2_1 = ep("u1"), ep("m2_1")
            _tt(nc, u1, s1a, s1a, Alu.mult)
            nc.vector.scalar_tensor_tensor(
                out=m2_1, in0=u1, scalar=-1.0 / N, in1=q1a, op0=Alu.mult, op1=Alu.add
            )
            u2, m2_2 = ep("u1"), ep("m2_2")
            _tt(nc, u2, s2a, s2a, Alu.mult)
            nc.vector.scalar_tensor_tensor(
                out=m2_2, in0=u2, scalar=-1.0 / N, in1=q2a, op0=Alu.mult, op1=Alu.add
            )
            u, num, w = ep("u1"), ep("num"), ep("w")
            _tt(nc, u, s1a, s2a, Alu.mult)
            nc.vector.scalar_tensor_tensor(
                out=num, in0=u, scalar=-1.0 / N, in1=s12a, op0=Alu.mult, op1=Alu.add
            )
            _tt(nc, w, m2_1, m2_2, Alu.mult)

            # cor = (num + n*EPS^2) / (sqrt(w) + (n-1)*EPS)
            sp, den, rden, cor = ep("sp"), ep("den"), ep("rden"), ep("cor")
            nc.scalar.activation(out=sp, in_=w, func=Act.Sqrt)
            nc.vector.tensor_scalar(
                out=den,
                in0=sp,
                scalar1=(N - 1) * EPS,
                scalar2=None,
                op0=Alu.add,
            )
            nc.vector.reciprocal(out=rden, in_=den)
            nc.vector.scalar_tensor_tensor(
                out=cor,
                in0=num,
                scalar=float(N) * EPS * EPS,
                in1=rden,
                op0=Alu.add,
                op1=Alu.mult,
            )
            c2, cor3 = ep("u1"), ep("cor3")
            _tt(nc, c2, cor, cor, Alu.mult)
            _tt(nc, cor3, c2, cor, Alu.mult)

            aa, lg, tl1 = ep("aa"), ep("lg"), ep("tl1")
            ln_bias = epi_pool.tile([128, 1], F32, tag="ln_bias", name="ln_bias")
            nc.vector.memset(ln_bias, 1.0 + EPS)
            nc.vector.scalar_tensor_tensor(
                out=aa, in0=cor3, scalar=-1.0, in1=cor3, op0=Alu.mult, op1=Alu.max
            )
            nc.scalar.activation(out=lg, in_=cor3, func=Act.Ln, bias=ln_bias)
            nc.vector.tensor_scalar(
                out=tl1,
                in0=lg,
                scalar1=-1.0,
                scalar2=float(np.log(2.0)),
                op0=Alu.mult,
                op1=Alu.add,
            )
            # tl2 = (2/N)*(1 - sum_min/se1)
            r1, um, tl2 = ep("r1"), ep("um"), ep("tl2")
            nc.vector.reciprocal(out=r1, in_=se1a)
            _tt(nc, um, statsT[:, 3, :], r1, Alu.mult)
            nc.vector.tensor_scalar(
                out=tl2,
                in0=um,
                scalar1=-2.0 / N,
                scalar2=2.0 / N,
                op0=Alu.mult,
                op1=Alu.add,
            )
            dd, pp, loss = ep("u1"), ep("pp"), ep("loss")
            _tt(nc, dd, tl1, tl2, Alu.subtract)
            _tt(nc, pp, aa, dd, Alu.mult)
            _tt(nc, loss, tl2, pp, Alu.add)

            part = epi_pool.tile([128, 1], F32, tag="part", name="part")
            nc.vector.reduce_sum(out=part, in_=loss, axis=mybir.AxisListType.X)
            nc.sync.dma_start(out=y[:, :], in_=part)

    split_waits(nc)
    return nc


_NC = None
_RUNNER = None


def _get_nc():
    global _NC
    if _NC is None:
        _NC = _build_program()
    return _NC


def _get_runner():
    """Compile the 8-core pjrt executable once and reuse across calls."""
    global _RUNNER
    if _RUNNER is not None:
        return _RUNNER
    import jax
    from jax.sharding import Mesh, PartitionSpec
    from jax.experimental.shard_map import shard_map
    from concourse.bass2jax import (
        _bass_exec_p,
        install_neuronx_cc_hook,
        partition_id_tensor,
    )

    install_neuronx_cc_hook()
    nc = _get_nc()
    partition_name = nc.partition_id_tensor.name if nc.partition_id_tensor else None
    in_names, out_names, out_avals, zero_outs = [], [], [], []
    for alloc in nc.m.functions[0].allocations:
        if not isinstance(alloc, mybir.MemoryLocationSet):
            continue
        name = alloc.memorylocations[0].name
        if alloc.kind == "ExternalInput":
            if name != partition_name:
                in_names.append(name)
        elif alloc.kind == "ExternalOutput":
            out_names.append(name)
            shape = tuple(alloc.tensor_shape)
            dtype = mybir.dt.np(alloc.dtype)
            out_avals.append(jax.core.ShapedArray(shape, dtype))
            zero_outs.append(np.zeros(shape, dtype))
    n_params = len(in_names)
    all_in_names = list(in_names) + out_names
    if partition_name is not None:
        all_in_names.append(partition_name)

    def _body(*args):
        operands = list(args)
        if partition_name is not None:
            operands.append(partition_id_tensor())
        outs = _bass_exec_p.bind(
            *operands,
            out_avals=tuple(out_avals),
            in_names=tuple(all_in_names),
            out_names=tuple(out_names),
            lowering_input_output_aliases=(),
            sim_require_finite=True,
            sim_require_nnan=True,
            nc=nc,
        )
        return tuple(outs)

    devices = jax.devices()[:N_CORES]
    mesh = Mesh(np.asarray(devices), ("core",))
    n_outs = len(out_names)
    in_specs = (PartitionSpec("core"),) * (n_params + n_outs)
    out_specs = (PartitionSpec("core"),) * n_outs
    sharded = jax.jit(
        shard_map(
            _body, mesh=mesh, in_specs=in_specs, out_specs=out_specs,
            check_rep=False,
        ),
        keep_unused=True,
    )
    zero_cat = [
        np.zeros((N_CORES * z.shape[0], *z.shape[1:]), z.dtype) for z in zero_outs
    ]

    def run(d1, d2):
        ins = {"d1": d1, "d2": d2}
        out = sharded(*(ins[nm] for nm in in_names), *zero_cat)
        y = np.asarray(out[out_names.index("y")])
        return y

    _RUNNER = run
    return _RUNNER


def kernel(distribution1, distribution2):
    d1 = np.ascontiguousarray(np.asarray(distribution1, dtype=np.float32))
    d2 = np.ascontiguousarray(np.asarray(distribution2, dtype=np.float32))
    assert d1.shape == (B, N) and d2.shape == (B, N)
    y = _get_runner()(d1, d2)  # [N_CORES*128, 1] partial sums
    return np.asarray([np.sum(y.astype(np.float64))], dtype=np.float32)
